# revision 3
# baseline (speedup 1.0000x reference)
"""MetapathAttentionLayer Trainium2 kernel.

Math (per node n):
    scores[n, m] = sum_d x[m, n, d] * W[d, m]
    att = softmax(relu(scores), axis=m)      (8 metapaths)
    out[n, :] = elu(sum_m att[n, m] * x[m, n, :])

Strategy: shard nodes across 8 cores (data parallel). Per core, natural
layout [nodes(part), d(free)] in bf16:
  - scores: DVE tensor_tensor mul vs replicated-W tile + tensor_scalar
    accum_out reductions (fused sum over d)
  - softmax: exp(relu(s)) == max(exp(s), 1); ACT Exp + DVE max/sum/recip
  - pooling: PE matmuls with diag(att_m) stationary (built by GPSIMD
    local_scatter), accumulating over m into PSUM
  - elu(x) = relu(x) + exp(min(x, 0)) - 1 composed on ACT
"""

import os
from contextlib import ExitStack

import numpy as np
import ml_dtypes

import concourse.bass as bass
import concourse.tile as tile
from concourse import bacc, mybir, library_config
import concourse.bass_utils as bass_utils

F32 = mybir.dt.float32
BF16 = mybir.dt.bfloat16
I16 = mybir.dt.int16
ALU = mybir.AluOpType
ACTF = mybir.ActivationFunctionType

NMETA = 8
N = 100000
D = 128
NCORES = 8
NC_RAW = N // NCORES          # 12500 nodes per core
CHUNK = 128                   # nodes per compute chunk (partition dim)
NC_PAD = 12544                # 98 chunks of 128
T_CHUNKS = 16                 # chunks per DMA T-tile (2048 nodes)
GROUP = 4                     # chunks per PSUM/elu group (psum bank = 512 f32)

# scores via fused tensor_tensor_reduce instead of TT + tensor_scalar accum
USE_TTR = False


def kernel_body(tc, out_d, x_d, wb_d, sidx_d, nc_pad=NC_PAD, t_chunks=T_CHUNKS):
    nc = tc.nc
    with ExitStack() as ctx:
        const = ctx.enter_context(tc.tile_pool(name="const", bufs=1))
        xpool = ctx.enter_context(tc.tile_pool(name="x", bufs=2))
        opool = ctx.enter_context(tc.tile_pool(name="o", bufs=2))
        ppool = ctx.enter_context(tc.tile_pool(name="prod", bufs=2))
        tpool = ctx.enter_context(tc.tile_pool(name="trash", bufs=2))
        spool = ctx.enter_context(tc.tile_pool(name="smalls", bufs=3))
        dpool = ctx.enter_context(tc.tile_pool(name="diag", bufs=3))
        epool = ctx.enter_context(tc.tile_pool(name="elu", bufs=2))
        psum = ctx.enter_context(tc.tile_pool(name="ps", bufs=4, space="PSUM"))

        wb = const.tile([128, NMETA * D], BF16)
        nc.sync.dma_start(wb[:], wb_d[:])
        sidx = const.tile([128, NMETA], I16)
        nc.sync.dma_start(sidx[:], sidx_d[:])
        nc.gpsimd.load_library(library_config.local_scatter)

        n0 = 0
        while n0 < nc_pad:
            ct = min(t_chunks, (nc_pad - n0) // CHUNK)  # chunks in this T-tile
            nt = ct * CHUNK                             # nodes in this T-tile

            # Load all 8 metapath slices for this tile of nodes.
            # node n = n0 + p*ct + c  ->  partition p, free chunk c
            X = xpool.tile([128, NMETA * nt], BF16)
            for m in range(NMETA):
                src = x_d[m, n0:n0 + nt, :].rearrange("(p c) d -> p (c d)", p=128)
                nc.sync.dma_start(X[:, m * nt:(m + 1) * nt], src)
            Xv = X[:].rearrange("p (m c d) -> p m c d", m=NMETA, c=ct)

            out_sb = opool.tile([128, nt], F32)

            for g0 in range(0, ct, GROUP):
                gl = min(GROUP, ct - g0)          # chunks in this group
                ps = psum.tile([128, GROUP * D], F32, tag="ps")
                scores = spool.tile([128, GROUP * NMETA], F32, tag="scores")

                for cg in range(gl):
                    c = g0 + cg
                    if USE_TTR:
                        tr = tpool.tile([128, D], BF16, tag="tr")
                        for m in range(NMETA):
                            nc.vector.tensor_tensor_reduce(
                                out=tr[:],
                                in0=Xv[:, m, c, :],
                                in1=wb[:].rearrange("p (m d) -> p m d", m=NMETA)[:, m, :],
                                scale=1.0,
                                scalar=0.0,
                                op0=ALU.mult,
                                op1=ALU.add,
                                accum_out=scores[:, cg * NMETA + m: cg * NMETA + m + 1],
                            )
                    else:
                        P = ppool.tile([128, NMETA * D], BF16, tag="P")
                        nc.vector.tensor_tensor(
                            out=P[:].rearrange("p (m d) -> p m d", m=NMETA),
                            in0=Xv[:, :, c, :],
                            in1=wb[:].rearrange("p (m d) -> p m d", m=NMETA),
                            op=ALU.mult,
                        )
                        tr = tpool.tile([128, D], BF16, tag="tr")
                        for m in range(NMETA):
                            nc.vector.tensor_scalar(
                                tr[:],
                                P[:, m * D:(m + 1) * D],
                                1.0,
                                None,
                                ALU.mult,
                                ALU.add,
                                accum_out=scores[:, cg * NMETA + m: cg * NMETA + m + 1],
                            )

                # softmax over m: att = e / sum(e), e = exp(relu(s)) = max(exp(s), 1)
                e_raw = spool.tile([128, GROUP * NMETA], F32, tag="eraw")
                nc.scalar.activation(e_raw[:, :gl * NMETA], scores[:, :gl * NMETA], ACTF.Exp)
                e_bf = spool.tile([128, GROUP * NMETA], BF16, tag="ebf")
                nc.vector.tensor_scalar(
                    e_bf[:, :gl * NMETA], e_raw[:, :gl * NMETA], 1.0, None, ALU.max)
                sums = spool.tile([128, GROUP], F32, tag="sums")
                nc.vector.tensor_reduce(
                    out=sums[:, :gl],
                    in_=e_bf[:, :gl * NMETA].rearrange("p (c m) -> p c m", m=NMETA),
                    axis=mybir.AxisListType.X,
                    op=ALU.add,
                )
                inv = spool.tile([128, GROUP], F32, tag="inv")
                nc.vector.reciprocal(inv[:, :gl], sums[:, :gl])

                for cg in range(gl):
                    c = g0 + cg
                    att = spool.tile([128, NMETA], BF16, tag="att")
                    nc.vector.tensor_scalar(
                        att[:], e_bf[:, cg * NMETA:(cg + 1) * NMETA],
                        inv[:, cg:cg + 1], None, ALU.mult)
                    diag = dpool.tile([128, NMETA * D], BF16, tag="diag")
                    nc.gpsimd.local_scatter(
                        diag[:], att[:], sidx[:],
                        channels=128, num_elems=NMETA * D, num_idxs=NMETA)
                    for m in range(NMETA):
                        nc.tensor.matmul(
                            out=ps[:, cg * D:(cg + 1) * D],
                            lhsT=diag[:, m * D:(m + 1) * D],
                            rhs=Xv[:, m, c, :],
                            start=(m == 0),
                            stop=(m == NMETA - 1),
                        )

                # elu(x) = relu(x) + exp(min(x,0)) - 1
                w = gl * D
                r = epool.tile([128, GROUP * D], F32, tag="r")
                nc.scalar.activation(r[:, :w], ps[:, :w], ACTF.Relu)
                t = epool.tile([128, GROUP * D], F32, tag="t")
                nc.scalar.activation(t[:, :w], ps[:, :w], ACTF.Relu, scale=-1.0)
                e2 = epool.tile([128, GROUP * D], F32, tag="e2")
                nc.scalar.activation(e2[:, :w], t[:, :w], ACTF.Exp, scale=-1.0)
                e2m1 = epool.tile([128, GROUP * D], F32, tag="e2m1")
                nc.vector.tensor_scalar(e2m1[:, :w], e2[:, :w], -1.0, None, ALU.add)
                nc.any.tensor_add(out_sb[:, g0 * D:g0 * D + w], e2m1[:, :w], r[:, :w])

            dst = out_d[n0:n0 + nt, :].rearrange("(p c) d -> p (c d)", p=128)
            nc.sync.dma_start(dst, out_sb[:])
            n0 += nt


def host_inputs(x_np, w_np, nc_pad=NC_PAD):
    """Build per-core input maps from full fp32 inputs."""
    in_maps = []
    wbig = np.ascontiguousarray(
        np.broadcast_to(w_np.T.reshape(1, NMETA * D), (128, NMETA * D))
    ).astype(ml_dtypes.bfloat16)
    sidx = (np.arange(NMETA)[None, :] * D + np.arange(128)[:, None]).astype(np.int16)
    nc_raw = x_np.shape[1] // NCORES
    for c in range(NCORES):
        xs = x_np[:, c * nc_raw:(c + 1) * nc_raw, :]
        xp = np.zeros((NMETA, nc_pad, D), dtype=ml_dtypes.bfloat16)
        xp[:, :nc_raw, :] = xs.astype(ml_dtypes.bfloat16)
        in_maps.append({"x": xp, "wb": wbig, "sidx": sidx})
    return in_maps


_CACHE = {}


def build():
    if "nc" in _CACHE:
        return _CACHE["nc"]
    nc = bacc.Bacc("TRN2", target_bir_lowering=False, debug=False,
                   num_devices=NCORES)
    x = nc.dram_tensor("x", [NMETA, NC_PAD, D], BF16, kind="ExternalInput").ap()
    wb = nc.dram_tensor("wb", [128, NMETA * D], BF16, kind="ExternalInput").ap()
    sidx = nc.dram_tensor("sidx", [128, NMETA], I16, kind="ExternalInput").ap()
    out = nc.dram_tensor("out", [NC_PAD, D], F32, kind="ExternalOutput").ap()
    with tile.TileContext(nc) as tc:
        kernel_body(tc, out, x, wb, sidx)
    nc.compile()
    _CACHE["nc"] = nc
    return nc


def run(input, W, trace=False, **trace_kwargs):
    x_np = np.asarray(input, dtype=np.float32)
    w_np = np.asarray(W, dtype=np.float32)
    nc = build()
    in_maps = host_inputs(x_np, w_np)
    res = bass_utils.run_bass_kernel_spmd(
        nc, in_maps, core_ids=list(range(NCORES)), trace=trace, **trace_kwargs)
    nc_raw = x_np.shape[1] // NCORES
    full = np.concatenate([res.results[c]["out"][:nc_raw] for c in range(NCORES)], axis=0)
    return full, res


def kernel(input, W):
    out, _ = run(input, W, trace=False)
    return out


# ---------------------------------------------------------------------------
# Timing harness (test-only): persistent jit over the bass_exec primitive so
# repeated executions reuse device-resident inputs.
# ---------------------------------------------------------------------------

def make_runner(nc):
    import jax
    from jax.experimental.shard_map import shard_map
    from jax.sharding import Mesh, PartitionSpec, NamedSharding
    from concourse import bass2jax as b2j

    b2j.install_neuronx_cc_hook()
    partition_name = nc.partition_id_tensor.name if nc.partition_id_tensor else None
    in_names, out_names, out_avals, zero_outs = [], [], [], []
    for alloc in nc.m.functions[0].allocations:
        if not isinstance(alloc, mybir.MemoryLocationSet):
            continue
        name = alloc.memorylocations[0].name
        if alloc.kind == "ExternalInput":
            if name != partition_name:
                in_names.append(name)
        elif alloc.kind == "ExternalOutput":
            out_names.append(name)
            shape = tuple(alloc.tensor_shape)
            dtype = mybir.dt.np(alloc.dtype)
            out_avals.append(jax.core.ShapedArray(shape, dtype))
            zero_outs.append(np.zeros(shape, dtype))
    n_params = len(in_names)
    n_outs = len(out_avals)
    all_names = in_names + out_names + ([partition_name] if partition_name else [])

    def _body(*args):
        operands = list(args)
        if partition_name is not None:
            operands.append(b2j.partition_id_tensor())
        outs = b2j._bass_exec_p.bind(
            *operands,
            out_avals=tuple(out_avals),
            in_names=tuple(all_names),
            out_names=tuple(out_names),
            lowering_input_output_aliases=(),
            sim_require_finite=True,
            sim_require_nnan=True,
            nc=nc,
        )
        return tuple(outs)

    devices = jax.devices()[:NCORES]
    mesh = Mesh(np.asarray(devices), ("core",))
    in_specs = (PartitionSpec("core"),) * (n_params + n_outs)
    out_specs = (PartitionSpec("core"),) * n_outs
    donate = tuple(range(n_params, n_params + n_outs))
    sharded = jax.jit(
        shard_map(_body, mesh=mesh, in_specs=in_specs, out_specs=out_specs,
                  check_rep=False),
        donate_argnums=donate, keep_unused=True)
    sharding = NamedSharding(mesh, PartitionSpec("core"))
    return sharded, in_names, zero_outs, sharding


def measure(input, W, reps=12):
    """Return (per-call wall times list, pipelined avg, outputs ok flag)."""
    import jax

    x_np = np.asarray(input, dtype=np.float32)
    w_np = np.asarray(W, dtype=np.float32)
    nc = build()
    in_maps = host_inputs(x_np, w_np)
    sharded, in_names, zero_outs, sharding = make_runner(nc)

    concat_in = [
        np.concatenate([in_maps[c][n] for c in range(NCORES)], axis=0)
        for n in in_names
    ]
    xs = [jax.device_put(a, sharding) for a in concat_in]
    zsets = []
    for _ in range(2 * reps + 1):
        zsets.append([
            jax.device_put(
                np.zeros((NCORES * z.shape[0], *z.shape[1:]), z.dtype), sharding)
            for z in zero_outs
        ])

    import time as _t
    o = sharded(*xs, *zsets[0])
    jax.block_until_ready(o)

    times = []
    for k in range(1, reps + 1):
        t0 = _t.perf_counter()
        o = sharded(*xs, *zsets[k])
        jax.block_until_ready(o)
        times.append(_t.perf_counter() - t0)

    # pipelined: issue all, sync once
    t0 = _t.perf_counter()
    outs = [sharded(*xs, *zsets[reps + 1 + k]) for k in range(reps)]
    jax.block_until_ready(outs)
    piped = (_t.perf_counter() - t0) / reps
    return times, piped


# revision 13
# speedup vs baseline: 12.8756x; 12.8756x over previous
"""MetapathAttentionLayer Trainium2 kernel.

Math (per node n):
    scores[n, m] = sum_d x[m, n, d] * W[d, m]
    att = softmax(relu(scores), axis=m)      (8 metapaths)
    out[n, :] = elu(sum_m att[n, m] * x[m, n, :])

Strategy: shard nodes across 8 cores (data parallel). Per core, natural
layout [nodes(part), d(free)] in bf16:
  - scores: DVE tensor_tensor mul vs replicated-W tile + tensor_scalar
    accum_out reductions (fused sum over d)
  - softmax: exp(relu(s)) == max(exp(s), 1); ACT Exp + DVE max/sum/recip
  - pooling: PE matmuls with diag(att_m) stationary (built by GPSIMD
    local_scatter / ACT tensor_tensor on identity blocks), accumulating
    over m into PSUM
  - elu(x) = relu(x) + exp(min(x, 0)) - 1 composed on ACT
"""

import os
from contextlib import ExitStack

import numpy as np
import ml_dtypes

import concourse.bass as bass
import concourse.tile as tile
from concourse import bacc, mybir, library_config
import concourse.bass_utils as bass_utils

F32 = mybir.dt.float32
BF16 = mybir.dt.bfloat16
I16 = mybir.dt.int16
ALU = mybir.AluOpType
ACTF = mybir.ActivationFunctionType

NMETA = 8
N = 100000
D = 128
NCORES = 8
NC_RAW = N // NCORES          # 12500 nodes per core
CHUNK = 128                   # nodes per compute chunk (partition dim)
NC_PAD = 12544                # 98 chunks of 128
T_CHUNKS = 16                 # chunks per DMA T-tile (2048 nodes)
GROUP = 4                     # chunks per PSUM/elu group (psum bank = 512 f32)

# tunables
DIAG_DVE_EVERY = 3   # every k-th chunk builds diag via DVE tensor_scalar (0=off)


def kernel_body(tc, out_d, x_d, wb_d, sidx_d, icat_d,
                nc_pad=NC_PAD, t_chunks=T_CHUNKS, reps=1,
                diag_dve_every=DIAG_DVE_EVERY):
    nc = tc.nc
    with ExitStack() as ctx:
        const = ctx.enter_context(tc.tile_pool(name="const", bufs=1))
        xpool = ctx.enter_context(tc.tile_pool(name="x", bufs=3))
        opool = ctx.enter_context(tc.tile_pool(name="o", bufs=2))
        ppool = ctx.enter_context(tc.tile_pool(name="prod", bufs=3))
        tpool = ctx.enter_context(tc.tile_pool(name="trash", bufs=2))
        spool = ctx.enter_context(tc.tile_pool(name="smalls", bufs=6))
        dpool = ctx.enter_context(tc.tile_pool(name="diag", bufs=6))
        epool = ctx.enter_context(tc.tile_pool(name="elu", bufs=3))
        psum = ctx.enter_context(tc.tile_pool(name="ps", bufs=6, space="PSUM"))

        wb = const.tile([128, NMETA * D], BF16)
        nc.sync.dma_start(wb[:], wb_d[:])
        sidx = const.tile([128, NMETA], I16)
        nc.sync.dma_start(sidx[:], sidx_d[:])
        icat = const.tile([128, NMETA * D], BF16)
        nc.sync.dma_start(icat[:], icat_d[:])
        nc.gpsimd.load_library(library_config.local_scatter)

        chunk_idx = 0
        for _rep in range(reps):
            n0 = 0
            while n0 < nc_pad:
                ct = min(t_chunks, (nc_pad - n0) // CHUNK)
                nt = ct * CHUNK

                # node n = n0 + p*ct + c  ->  partition p, free chunk c
                X = xpool.tile([128, NMETA * nt], BF16, tag="X")
                for m in range(NMETA):
                    src = x_d[m, n0:n0 + nt, :].rearrange(
                        "(p c) d -> p (c d)", p=128)
                    nc.sync.dma_start(X[:, m * nt:(m + 1) * nt], src)
                Xv = X[:].rearrange("p (m c d) -> p m c d", m=NMETA, c=ct)

                out_sb = opool.tile([128, nt], F32, tag="osb")

                for g0 in range(0, ct, GROUP):
                    gl = min(GROUP, ct - g0)
                    ps = psum.tile([128, GROUP * D], F32, tag="ps")
                    scores = spool.tile([128, GROUP * NMETA], F32, tag="scores")

                    # one batched multiply for the whole group of chunks
                    P = ppool.tile([128, NMETA * GROUP * D], BF16, tag="P")
                    Pv = P[:].rearrange("p (m c d) -> p m c d", m=NMETA, c=GROUP)
                    nc.vector.tensor_tensor(
                        out=Pv[:, :, :gl, :],
                        in0=Xv[:, :, g0:g0 + gl, :],
                        in1=wb[:].rearrange("p (m d) -> p m d", m=NMETA)
                              .unsqueeze(2).broadcast_to([128, NMETA, gl, D]),
                        op=ALU.mult,
                    )
                    tr = tpool.tile([128, D], BF16, tag="tr")
                    for cg in range(gl):
                        for m in range(NMETA):
                            nc.vector.tensor_scalar(
                                tr[:],
                                Pv[:, m, cg, :],
                                1.0,
                                None,
                                ALU.mult,
                                ALU.add,
                                accum_out=scores[:, cg * NMETA + m:
                                                 cg * NMETA + m + 1],
                            )

                    # softmax over m: att = e/sum(e), e = exp(relu(s)) = max(exp(s),1)
                    e_raw = spool.tile([128, GROUP * NMETA], F32, tag="eraw")
                    nc.scalar.activation(
                        e_raw[:, :gl * NMETA], scores[:, :gl * NMETA], ACTF.Exp)
                    e_bf = spool.tile([128, GROUP * NMETA], BF16, tag="ebf")
                    nc.vector.tensor_scalar(
                        e_bf[:, :gl * NMETA], e_raw[:, :gl * NMETA],
                        1.0, None, ALU.max)
                    sums = spool.tile([128, GROUP], F32, tag="sums")
                    nc.vector.tensor_reduce(
                        out=sums[:, :gl],
                        in_=e_bf[:, :gl * NMETA].rearrange(
                            "p (c m) -> p c m", m=NMETA),
                        axis=mybir.AxisListType.X,
                        op=ALU.add,
                    )
                    inv = spool.tile([128, GROUP], F32, tag="inv")
                    nc.vector.reciprocal(inv[:, :gl], sums[:, :gl])

                    for cg in range(gl):
                        c = g0 + cg
                        diag = dpool.tile([128, NMETA * D], BF16, tag="diag")
                        use_dve = (diag_dve_every and
                                   chunk_idx % diag_dve_every == 0)
                        if use_dve:
                            att_f = spool.tile([128, NMETA], F32, tag="attf")
                            nc.vector.tensor_scalar(
                                att_f[:], e_bf[:, cg * NMETA:(cg + 1) * NMETA],
                                inv[:, cg:cg + 1], None, ALU.mult)
                            for m in range(NMETA):
                                nc.vector.tensor_scalar(
                                    diag[:, m * D:(m + 1) * D],
                                    icat[:, m * D:(m + 1) * D],
                                    att_f[:, m:m + 1], None, ALU.mult)
                        else:
                            att = spool.tile([128, NMETA], BF16, tag="att")
                            nc.vector.tensor_scalar(
                                att[:], e_bf[:, cg * NMETA:(cg + 1) * NMETA],
                                inv[:, cg:cg + 1], None, ALU.mult)
                            nc.gpsimd.local_scatter(
                                diag[:], att[:], sidx[:],
                                channels=128, num_elems=NMETA * D,
                                num_idxs=NMETA)
                        for m in range(NMETA):
                            nc.tensor.matmul(
                                out=ps[:, cg * D:(cg + 1) * D],
                                lhsT=diag[:, m * D:(m + 1) * D],
                                rhs=Xv[:, m, c, :],
                                start=(m == 0),
                                stop=(m == NMETA - 1),
                            )
                        chunk_idx += 1

                    # elu(x) = relu(x) + exp(min(x,0)) - 1
                    w = gl * D
                    r = epool.tile([128, GROUP * D], F32, tag="r")
                    nc.scalar.activation(r[:, :w], ps[:, :w], ACTF.Relu)
                    t = epool.tile([128, GROUP * D], F32, tag="t")
                    nc.scalar.activation(t[:, :w], ps[:, :w], ACTF.Relu,
                                         scale=-1.0)
                    e2 = epool.tile([128, GROUP * D], F32, tag="e2")
                    nc.scalar.activation(e2[:, :w], t[:, :w], ACTF.Exp,
                                         scale=-1.0)
                    # out = (e2 + (-1)) + r  in one fused DVE op
                    nc.vector.scalar_tensor_tensor(
                        out=out_sb[:, g0 * D:g0 * D + w],
                        in0=e2[:, :w], scalar=-1.0, in1=r[:, :w],
                        op0=ALU.add, op1=ALU.add)

                dsto = out_d[n0:n0 + nt, :].rearrange("(p c) d -> p (c d)", p=128)
                nc.sync.dma_start(dsto, out_sb[:])
                n0 += nt


def host_inputs(x_np, w_np, nc_pad=NC_PAD):
    """Build per-core input maps from full fp32 inputs."""
    in_maps = []
    wbig = np.ascontiguousarray(
        np.broadcast_to(w_np.T.reshape(1, NMETA * D), (128, NMETA * D))
    ).astype(ml_dtypes.bfloat16)
    sidx = (np.arange(NMETA)[None, :] * D + np.arange(128)[:, None]).astype(np.int16)
    icat = np.ascontiguousarray(
        np.tile(np.eye(128, dtype=np.float32), (1, NMETA))
    ).astype(ml_dtypes.bfloat16)
    nc_raw = x_np.shape[1] // NCORES
    for c in range(NCORES):
        xs = x_np[:, c * nc_raw:(c + 1) * nc_raw, :]
        xp = np.zeros((NMETA, nc_pad, D), dtype=ml_dtypes.bfloat16)
        xp[:, :nc_raw, :] = xs.astype(ml_dtypes.bfloat16)
        in_maps.append({"x": xp, "wb": wbig, "sidx": sidx, "icat": icat})
    return in_maps


_CACHE = {}


def build(reps=1, **kw):
    key = (reps, tuple(sorted(kw.items())))
    if key in _CACHE:
        return _CACHE[key]
    nc = bacc.Bacc("TRN2", target_bir_lowering=False, debug=False,
                   num_devices=NCORES)
    x = nc.dram_tensor("x", [NMETA, NC_PAD, D], BF16, kind="ExternalInput").ap()
    wb = nc.dram_tensor("wb", [128, NMETA * D], BF16, kind="ExternalInput").ap()
    sidx = nc.dram_tensor("sidx", [128, NMETA], I16, kind="ExternalInput").ap()
    icat = nc.dram_tensor("icat", [128, NMETA * D], BF16, kind="ExternalInput").ap()
    out = nc.dram_tensor("out", [NC_PAD, D], F32, kind="ExternalOutput").ap()
    with tile.TileContext(nc) as tc:
        kernel_body(tc, out, x, wb, sidx, icat, reps=reps, **kw)
    nc.compile()
    _CACHE[key] = nc
    return nc


def run(input, W, trace=False, **trace_kwargs):
    x_np = np.asarray(input, dtype=np.float32)
    w_np = np.asarray(W, dtype=np.float32)
    nc = build()
    in_maps = host_inputs(x_np, w_np)
    res = bass_utils.run_bass_kernel_spmd(
        nc, in_maps, core_ids=list(range(NCORES)), trace=trace, **trace_kwargs)
    nc_raw = x_np.shape[1] // NCORES
    full = np.concatenate(
        [res.results[c]["out"][:nc_raw] for c in range(NCORES)], axis=0)
    return full, res


def kernel(input, W):
    out, _ = run(input, W, trace=False)
    return out


# ---------------------------------------------------------------------------
# Timing harness (test-only): persistent jit over the bass_exec primitive so
# repeated executions reuse device-resident inputs. HW kernel time is derived
# from the slope between an R-repeat NEFF and the 1-repeat NEFF.
# ---------------------------------------------------------------------------

def make_runner(nc):
    import jax
    from jax.experimental.shard_map import shard_map
    from jax.sharding import Mesh, PartitionSpec, NamedSharding
    from concourse import bass2jax as b2j

    b2j.install_neuronx_cc_hook()
    partition_name = nc.partition_id_tensor.name if nc.partition_id_tensor else None
    in_names, out_names, out_avals, zero_outs = [], [], [], []
    for alloc in nc.m.functions[0].allocations:
        if not isinstance(alloc, mybir.MemoryLocationSet):
            continue
        name = alloc.memorylocations[0].name
        if alloc.kind == "ExternalInput":
            if name != partition_name:
                in_names.append(name)
        elif alloc.kind == "ExternalOutput":
            out_names.append(name)
            shape = tuple(alloc.tensor_shape)
            dtype = mybir.dt.np(alloc.dtype)
            out_avals.append(jax.core.ShapedArray(shape, dtype))
            zero_outs.append(np.zeros(shape, dtype))
    n_params = len(in_names)
    n_outs = len(out_avals)
    all_names = in_names + out_names + ([partition_name] if partition_name else [])

    def _body(*args):
        operands = list(args)
        if partition_name is not None:
            operands.append(b2j.partition_id_tensor())
        outs = b2j._bass_exec_p.bind(
            *operands,
            out_avals=tuple(out_avals),
            in_names=tuple(all_names),
            out_names=tuple(out_names),
            lowering_input_output_aliases=(),
            sim_require_finite=True,
            sim_require_nnan=True,
            nc=nc,
        )
        return tuple(outs)

    devices = jax.devices()[:NCORES]
    mesh = Mesh(np.asarray(devices), ("core",))
    in_specs = (PartitionSpec("core"),) * (n_params + n_outs)
    out_specs = (PartitionSpec("core"),) * n_outs
    donate = tuple(range(n_params, n_params + n_outs))
    sharded = jax.jit(
        shard_map(_body, mesh=mesh, in_specs=in_specs, out_specs=out_specs,
                  check_rep=False),
        donate_argnums=donate, keep_unused=True)
    sharding = NamedSharding(mesh, PartitionSpec("core"))
    return sharded, in_names, zero_outs, sharding


class _TimedRunner:
    def __init__(self, nc, in_maps):
        import jax
        self.jax = jax
        sharded, in_names, zero_outs, sharding = make_runner(nc)
        self.sharded = sharded
        concat_in = [
            np.concatenate([in_maps[c][n] for c in range(NCORES)], axis=0)
            for n in in_names
        ]
        self.xs = [jax.device_put(a, sharding) for a in concat_in]
        self.zero_outs = zero_outs
        self.sharding = sharding

    def _zset(self):
        return [
            self.jax.device_put(
                np.zeros((NCORES * z.shape[0], *z.shape[1:]), z.dtype),
                self.sharding)
            for z in self.zero_outs
        ]

    def piped(self, reps):
        import time as _t
        zsets = [self._zset() for _ in range(reps + 1)]
        self.jax.block_until_ready(zsets)
        self.jax.block_until_ready(self.xs)
        o = self.sharded(*self.xs, *zsets[0])
        self.jax.block_until_ready(o)
        _ = self.jax.device_get(o[0])
        t0 = _t.perf_counter()
        outs = [self.sharded(*self.xs, *zsets[1 + k]) for k in range(reps)]
        self.jax.block_until_ready(outs)
        # force true device completion: fetch the last output's bytes
        _ = self.jax.device_get(outs[-1][0])
        return (_t.perf_counter() - t0) / reps


def measure(input, W, reps=12, neff_reps=9, rounds=4, **kw):
    """Estimate per-iteration HW time via multi-repeat NEFF slope.

    Interleaves rounds of (1-repeat NEFF, R-repeat NEFF) piped timings and
    takes the min across rounds for each to reject dispatch-overhead noise.
    """
    x_np = np.asarray(input, dtype=np.float32)
    w_np = np.asarray(W, dtype=np.float32)
    in_maps = host_inputs(x_np, w_np)

    nc1 = build(reps=1, **kw)
    ncr = build(reps=neff_reps, **kw)
    r1 = _TimedRunner(nc1, in_maps)
    rr = _TimedRunner(ncr, in_maps)
    t1s, trs = [], []
    for _ in range(rounds):
        t1s.append(r1.piped(reps))
        trs.append(rr.piped(reps))
    t1, tr = min(t1s), min(trs)
    slope = (tr - t1) / (neff_reps - 1)
    return t1, tr, slope, t1s, trs


# revision 14
# speedup vs baseline: 44.5079x; 3.4568x over previous
"""MetapathAttentionLayer Trainium2 kernel.

Math (per node n):
    scores[n, m] = sum_d x[m, n, d] * W[d, m]
    att = softmax(relu(scores), axis=m)      (8 metapaths)
    out[n, :] = elu(sum_m att[n, m] * x[m, n, :])

Strategy: shard nodes across 8 cores (data parallel). Per core, natural
layout [nodes(part), d(free)] in bf16:
  - scores: DVE tensor_tensor mul vs replicated-W tile + tensor_scalar
    accum_out reductions (fused sum over d)
  - softmax: exp(relu(s)) == max(exp(s), 1); ACT Exp + DVE max/sum/recip
  - pooling: PE matmuls with diag(att_m) stationary (built by GPSIMD
    local_scatter / ACT tensor_tensor on identity blocks), accumulating
    over m into PSUM
  - elu(x) = relu(x) + exp(min(x, 0)) - 1 composed on ACT
"""

import os
from contextlib import ExitStack

import numpy as np
import ml_dtypes

import concourse.bass as bass
import concourse.tile as tile
from concourse import bacc, mybir, library_config
import concourse.bass_utils as bass_utils

F32 = mybir.dt.float32
BF16 = mybir.dt.bfloat16
I16 = mybir.dt.int16
ALU = mybir.AluOpType
ACTF = mybir.ActivationFunctionType

NMETA = 8
N = 100000
D = 128
NCORES = 8
NC_RAW = N // NCORES          # 12500 nodes per core
CHUNK = 128                   # nodes per compute chunk (partition dim)
NC_PAD = 12544                # 98 chunks of 128
T_CHUNKS = 8                  # chunks per DMA T-tile (1024 nodes)
GROUP = 4                     # chunks per PSUM/elu group (psum bank = 512 f32)

# tunables
DIAG_DVE_EVERY = 3   # every k-th chunk builds diag via DVE tensor_scalar (0=off)


def kernel_body(tc, out_d, x_d, wb_d, sidx_d, icat_d,
                nc_pad=NC_PAD, t_chunks=T_CHUNKS, reps=1,
                diag_dve_every=DIAG_DVE_EVERY):
    nc = tc.nc
    with ExitStack() as ctx:
        const = ctx.enter_context(tc.tile_pool(name="const", bufs=1))
        xpool = ctx.enter_context(tc.tile_pool(name="x", bufs=3))
        opool = ctx.enter_context(tc.tile_pool(name="o", bufs=2))
        ppool = ctx.enter_context(tc.tile_pool(name="prod", bufs=3))
        tpool = ctx.enter_context(tc.tile_pool(name="trash", bufs=2))
        spool = ctx.enter_context(tc.tile_pool(name="smalls", bufs=6))
        dpool = ctx.enter_context(tc.tile_pool(name="diag", bufs=6))
        epool = ctx.enter_context(tc.tile_pool(name="elu", bufs=3))
        psum = ctx.enter_context(tc.tile_pool(name="ps", bufs=6, space="PSUM"))

        wb = const.tile([128, NMETA * D], BF16)
        nc.sync.dma_start(wb[:], wb_d[:])
        sidx = const.tile([128, NMETA], I16)
        nc.sync.dma_start(sidx[:], sidx_d[:])
        icat = const.tile([128, NMETA * D], BF16)
        nc.sync.dma_start(icat[:], icat_d[:])
        nc.gpsimd.load_library(library_config.local_scatter)

        chunk_idx = 0
        for _rep in range(reps):
            n0 = 0
            while n0 < nc_pad:
                ct = min(t_chunks, (nc_pad - n0) // CHUNK)
                nt = ct * CHUNK

                # node n = n0 + p*ct + c  ->  partition p, free chunk c
                X = xpool.tile([128, NMETA * nt], BF16, tag="X")
                for m in range(NMETA):
                    src = x_d[m, n0:n0 + nt, :].rearrange(
                        "(p c) d -> p (c d)", p=128)
                    nc.sync.dma_start(X[:, m * nt:(m + 1) * nt], src)
                Xv = X[:].rearrange("p (m c d) -> p m c d", m=NMETA, c=ct)

                out_sb = opool.tile([128, nt], F32, tag="osb")

                for g0 in range(0, ct, GROUP):
                    gl = min(GROUP, ct - g0)
                    ps = psum.tile([128, GROUP * D], F32, tag="ps")
                    scores = spool.tile([128, GROUP * NMETA], F32, tag="scores")

                    # one batched multiply for the whole group of chunks
                    P = ppool.tile([128, NMETA * GROUP * D], BF16, tag="P")
                    Pv = P[:].rearrange("p (m c d) -> p m c d", m=NMETA, c=GROUP)
                    nc.vector.tensor_tensor(
                        out=Pv[:, :, :gl, :],
                        in0=Xv[:, :, g0:g0 + gl, :],
                        in1=wb[:].rearrange("p (m d) -> p m d", m=NMETA)
                              .unsqueeze(2).broadcast_to([128, NMETA, gl, D]),
                        op=ALU.mult,
                    )
                    tr = tpool.tile([128, D], BF16, tag="tr")
                    for cg in range(gl):
                        for m in range(NMETA):
                            nc.vector.tensor_scalar(
                                tr[:],
                                Pv[:, m, cg, :],
                                1.0,
                                None,
                                ALU.mult,
                                ALU.add,
                                accum_out=scores[:, cg * NMETA + m:
                                                 cg * NMETA + m + 1],
                            )

                    # softmax over m: att = e/sum(e), e = exp(relu(s)) = max(exp(s),1)
                    e_raw = spool.tile([128, GROUP * NMETA], F32, tag="eraw")
                    nc.scalar.activation(
                        e_raw[:, :gl * NMETA], scores[:, :gl * NMETA], ACTF.Exp)
                    e_bf = spool.tile([128, GROUP * NMETA], BF16, tag="ebf")
                    nc.vector.tensor_scalar(
                        e_bf[:, :gl * NMETA], e_raw[:, :gl * NMETA],
                        1.0, None, ALU.max)
                    sums = spool.tile([128, GROUP], F32, tag="sums")
                    nc.vector.tensor_reduce(
                        out=sums[:, :gl],
                        in_=e_bf[:, :gl * NMETA].rearrange(
                            "p (c m) -> p c m", m=NMETA),
                        axis=mybir.AxisListType.X,
                        op=ALU.add,
                    )
                    inv = spool.tile([128, GROUP], F32, tag="inv")
                    nc.vector.reciprocal(inv[:, :gl], sums[:, :gl])

                    for cg in range(gl):
                        c = g0 + cg
                        diag = dpool.tile([128, NMETA * D], BF16, tag="diag")
                        use_dve = (diag_dve_every and
                                   chunk_idx % diag_dve_every == 0)
                        if use_dve:
                            att_f = spool.tile([128, NMETA], F32, tag="attf")
                            nc.vector.tensor_scalar(
                                att_f[:], e_bf[:, cg * NMETA:(cg + 1) * NMETA],
                                inv[:, cg:cg + 1], None, ALU.mult)
                            for m in range(NMETA):
                                nc.vector.tensor_scalar(
                                    diag[:, m * D:(m + 1) * D],
                                    icat[:, m * D:(m + 1) * D],
                                    att_f[:, m:m + 1], None, ALU.mult)
                        else:
                            att = spool.tile([128, NMETA], BF16, tag="att")
                            nc.vector.tensor_scalar(
                                att[:], e_bf[:, cg * NMETA:(cg + 1) * NMETA],
                                inv[:, cg:cg + 1], None, ALU.mult)
                            nc.gpsimd.local_scatter(
                                diag[:], att[:], sidx[:],
                                channels=128, num_elems=NMETA * D,
                                num_idxs=NMETA)
                        for m in range(NMETA):
                            nc.tensor.matmul(
                                out=ps[:, cg * D:(cg + 1) * D],
                                lhsT=diag[:, m * D:(m + 1) * D],
                                rhs=Xv[:, m, c, :],
                                start=(m == 0),
                                stop=(m == NMETA - 1),
                            )
                        chunk_idx += 1

                    # elu(x) = relu(x) + exp(min(x,0)) - 1
                    w = gl * D
                    r = epool.tile([128, GROUP * D], F32, tag="r")
                    nc.scalar.activation(r[:, :w], ps[:, :w], ACTF.Relu)
                    t = epool.tile([128, GROUP * D], F32, tag="t")
                    nc.scalar.activation(t[:, :w], ps[:, :w], ACTF.Relu,
                                         scale=-1.0)
                    e2 = epool.tile([128, GROUP * D], F32, tag="e2")
                    nc.scalar.activation(e2[:, :w], t[:, :w], ACTF.Exp,
                                         scale=-1.0)
                    # out = (e2 + (-1)) + r  in one fused DVE op
                    nc.vector.scalar_tensor_tensor(
                        out=out_sb[:, g0 * D:g0 * D + w],
                        in0=e2[:, :w], scalar=-1.0, in1=r[:, :w],
                        op0=ALU.add, op1=ALU.add)

                dsto = out_d[n0:n0 + nt, :].rearrange("(p c) d -> p (c d)", p=128)
                nc.sync.dma_start(dsto, out_sb[:])
                n0 += nt


def host_inputs(x_np, w_np, nc_pad=NC_PAD):
    """Build per-core input maps from full fp32 inputs."""
    in_maps = []
    wbig = np.ascontiguousarray(
        np.broadcast_to(w_np.T.reshape(1, NMETA * D), (128, NMETA * D))
    ).astype(ml_dtypes.bfloat16)
    sidx = (np.arange(NMETA)[None, :] * D + np.arange(128)[:, None]).astype(np.int16)
    icat = np.ascontiguousarray(
        np.tile(np.eye(128, dtype=np.float32), (1, NMETA))
    ).astype(ml_dtypes.bfloat16)
    nc_raw = x_np.shape[1] // NCORES
    for c in range(NCORES):
        xs = x_np[:, c * nc_raw:(c + 1) * nc_raw, :]
        xp = np.zeros((NMETA, nc_pad, D), dtype=ml_dtypes.bfloat16)
        xp[:, :nc_raw, :] = xs.astype(ml_dtypes.bfloat16)
        in_maps.append({"x": xp, "wb": wbig, "sidx": sidx, "icat": icat})
    return in_maps


_CACHE = {}


def build(reps=1, **kw):
    key = (reps, tuple(sorted(kw.items())))
    if key in _CACHE:
        return _CACHE[key]
    nc = bacc.Bacc("TRN2", target_bir_lowering=False, debug=False,
                   num_devices=NCORES)
    x = nc.dram_tensor("x", [NMETA, NC_PAD, D], BF16, kind="ExternalInput").ap()
    wb = nc.dram_tensor("wb", [128, NMETA * D], BF16, kind="ExternalInput").ap()
    sidx = nc.dram_tensor("sidx", [128, NMETA], I16, kind="ExternalInput").ap()
    icat = nc.dram_tensor("icat", [128, NMETA * D], BF16, kind="ExternalInput").ap()
    out = nc.dram_tensor("out", [NC_PAD, D], F32, kind="ExternalOutput").ap()
    with tile.TileContext(nc) as tc:
        kernel_body(tc, out, x, wb, sidx, icat, reps=reps, **kw)
    nc.compile()
    _CACHE[key] = nc
    return nc


def run(input, W, trace=False, **trace_kwargs):
    x_np = np.asarray(input, dtype=np.float32)
    w_np = np.asarray(W, dtype=np.float32)
    nc = build()
    in_maps = host_inputs(x_np, w_np)
    res = bass_utils.run_bass_kernel_spmd(
        nc, in_maps, core_ids=list(range(NCORES)), trace=trace, **trace_kwargs)
    nc_raw = x_np.shape[1] // NCORES
    full = np.concatenate(
        [res.results[c]["out"][:nc_raw] for c in range(NCORES)], axis=0)
    return full, res


def kernel(input, W):
    out, _ = run(input, W, trace=False)
    return out


# ---------------------------------------------------------------------------
# Timing harness (test-only): persistent jit over the bass_exec primitive so
# repeated executions reuse device-resident inputs. HW kernel time is derived
# from the slope between an R-repeat NEFF and the 1-repeat NEFF.
# ---------------------------------------------------------------------------

def make_runner(nc):
    import jax
    from jax.experimental.shard_map import shard_map
    from jax.sharding import Mesh, PartitionSpec, NamedSharding
    from concourse import bass2jax as b2j

    b2j.install_neuronx_cc_hook()
    partition_name = nc.partition_id_tensor.name if nc.partition_id_tensor else None
    in_names, out_names, out_avals, zero_outs = [], [], [], []
    for alloc in nc.m.functions[0].allocations:
        if not isinstance(alloc, mybir.MemoryLocationSet):
            continue
        name = alloc.memorylocations[0].name
        if alloc.kind == "ExternalInput":
            if name != partition_name:
                in_names.append(name)
        elif alloc.kind == "ExternalOutput":
            out_names.append(name)
            shape = tuple(alloc.tensor_shape)
            dtype = mybir.dt.np(alloc.dtype)
            out_avals.append(jax.core.ShapedArray(shape, dtype))
            zero_outs.append(np.zeros(shape, dtype))
    n_params = len(in_names)
    n_outs = len(out_avals)
    all_names = in_names + out_names + ([partition_name] if partition_name else [])

    def _body(*args):
        operands = list(args)
        if partition_name is not None:
            operands.append(b2j.partition_id_tensor())
        outs = b2j._bass_exec_p.bind(
            *operands,
            out_avals=tuple(out_avals),
            in_names=tuple(all_names),
            out_names=tuple(out_names),
            lowering_input_output_aliases=(),
            sim_require_finite=True,
            sim_require_nnan=True,
            nc=nc,
        )
        return tuple(outs)

    devices = jax.devices()[:NCORES]
    mesh = Mesh(np.asarray(devices), ("core",))
    in_specs = (PartitionSpec("core"),) * (n_params + n_outs)
    out_specs = (PartitionSpec("core"),) * n_outs
    donate = tuple(range(n_params, n_params + n_outs))
    sharded = jax.jit(
        shard_map(_body, mesh=mesh, in_specs=in_specs, out_specs=out_specs,
                  check_rep=False),
        donate_argnums=donate, keep_unused=True)
    sharding = NamedSharding(mesh, PartitionSpec("core"))
    return sharded, in_names, zero_outs, sharding


class _TimedRunner:
    def __init__(self, nc, in_maps):
        import jax
        self.jax = jax
        sharded, in_names, zero_outs, sharding = make_runner(nc)
        self.sharded = sharded
        concat_in = [
            np.concatenate([in_maps[c][n] for c in range(NCORES)], axis=0)
            for n in in_names
        ]
        self.xs = [jax.device_put(a, sharding) for a in concat_in]
        self.zero_outs = zero_outs
        self.sharding = sharding

    def _zset(self):
        return [
            self.jax.device_put(
                np.zeros((NCORES * z.shape[0], *z.shape[1:]), z.dtype),
                self.sharding)
            for z in self.zero_outs
        ]

    def piped(self, reps):
        import time as _t
        zsets = [self._zset() for _ in range(reps + 1)]
        self.jax.block_until_ready(zsets)
        self.jax.block_until_ready(self.xs)
        o = self.sharded(*self.xs, *zsets[0])
        self.jax.block_until_ready(o)
        _ = self.jax.device_get(o[0])
        t0 = _t.perf_counter()
        outs = [self.sharded(*self.xs, *zsets[1 + k]) for k in range(reps)]
        self.jax.block_until_ready(outs)
        # force true device completion: fetch the last output's bytes
        _ = self.jax.device_get(outs[-1][0])
        return (_t.perf_counter() - t0) / reps


def measure(input, W, reps=12, neff_reps=9, rounds=4, **kw):
    """Estimate per-iteration HW time via multi-repeat NEFF slope.

    Interleaves rounds of (1-repeat NEFF, R-repeat NEFF) piped timings and
    takes the min across rounds for each to reject dispatch-overhead noise.
    """
    x_np = np.asarray(input, dtype=np.float32)
    w_np = np.asarray(W, dtype=np.float32)
    in_maps = host_inputs(x_np, w_np)

    nc1 = build(reps=1, **kw)
    ncr = build(reps=neff_reps, **kw)
    r1 = _TimedRunner(nc1, in_maps)
    rr = _TimedRunner(ncr, in_maps)
    t1s, trs = [], []
    for _ in range(rounds):
        t1s.append(r1.piped(reps))
        trs.append(rr.piped(reps))
    t1, tr = min(t1s), min(trs)
    slope = (tr - t1) / (neff_reps - 1)
    return t1, tr, slope, t1s, trs
